# revision 7
# baseline (speedup 1.0000x reference)
"""Trainium2 Bass kernel for nn_AugmentedConv (conv branch + conv-attention branch).

Full-input contract: kernel(**inputs) takes the complete unsharded inputs and
returns the full (8, 512, 2048) output. Internally: data-parallel over batch
across 8 NeuronCores; each core runs the whole module for one batch element.

v2: PE tile_position packing + split exp across ScalarE/DVE.
 - logits: 4 heads concurrent via 32-row tile_position groups (contract=32).
 - attnV: 2 heads per PSUM bank via 64-col tile_position groups; each head
   block in vt is 64 wide (32 v dims + ones col for the softmax denominator).
 - exp: ScalarE exact exp for half the tiles; DVE Schraudolph uint16 bit-trick
   (out = bits(round(x*a+b)) viewed as fp16, saturating: negatives -> +0) for
   the rest. ~3% per-element error on those tiles; cancels largely in softmax.

Hardcoded problem shapes: B=8, C=256, W=2048, DK=DV=256, NH=8, KS=3, pad=1.
"""

import numpy as np

import concourse.bacc as bacc
import concourse.mybir as mybir
import concourse.tile as tile
from concourse import bass_utils
from concourse.masks import make_identity

F32 = mybir.dt.float32
F16 = mybir.dt.float16
U16 = mybir.dt.uint16
ESHIFT = -4.0   # exp(x + ESHIFT): keeps fp16 exp in range; cancels in softmax ratio

C = 256          # input channels
W = 2048         # sequence length
OC = 1024        # combined conv output channels: [conv_out 256 | q 256 | k 256 | v 256]
NH = 8
DKH = 32         # head dim (dk and dv per head)
QSCALE = float(DKH) ** -0.5
NCT = C // 128   # input-channel tiles (2)
NWT = W // 512   # 512-wide w tiles (4)
NMT = W // 128   # 128-wide w tiles (16)

LOG2E = 1.4426950408889634
SCHR_A = 1024.0 * LOG2E
SCHR_B = ESHIFT * SCHR_A + 15 * 1024 - 46.0

# knobs
DVE_EXP_MOD = 3      # DVE handles lg23 when (mstep % DVE_EXP_MOD) != DVE_EXP_MOD-1
CONV_EVERY = 7       # emit one conv unit every N m-steps
PENDING_PER_STEP = 3 # deferred normalize items popped per m-step


def build_nc():
    nc = bacc.Bacc("TRN2", target_bir_lowering=False, debug=False)

    x_d = nc.dram_tensor("x", [C, W], F16, kind="ExternalInput")
    wt_d = nc.dram_tensor("wt", [3, C, OC], F16, kind="ExternalInput")     # wt[k,c,oc]
    ball_d = nc.dram_tensor("b_all", [OC], F32, kind="ExternalInput")      # q part pre-scaled
    watt_d = nc.dram_tensor("watT", [C, C], F16, kind="ExternalInput")     # w_attn.T (c,o)
    bat_d = nc.dram_tensor("bat", [C], F32, kind="ExternalInput")
    out_d = nc.dram_tensor("out", [2 * C, W], F32, kind="ExternalOutput")
    # attention output staged in [h, w, d] order; the module's faithful
    # (NH,W,dvh)->(256,W) reshape is then a contiguous view of this buffer.
    ahwd_d = nc.dram_tensor("attn_hwd", [NH, W, DKH], F16)

    with tile.TileContext(nc) as tc:
        import contextlib
        with contextlib.ExitStack() as ctx:
            singles = ctx.enter_context(tc.tile_pool(name="singles", bufs=1))
            xp = ctx.enter_context(tc.tile_pool(name="xp", bufs=NCT))
            wtp = ctx.enter_context(tc.tile_pool(name="wtp", bufs=3 * NCT))
            qkp = ctx.enter_context(tc.tile_pool(name="qkp", bufs=4))
            vtp = ctx.enter_context(tc.tile_pool(name="vtp", bufs=NMT))
            ep = ctx.enter_context(tc.tile_pool(name="ep", bufs=6))
            stage = ctx.enter_context(tc.tile_pool(name="stage", bufs=4))
            norm = ctx.enter_context(tc.tile_pool(name="norm", bufs=10))
            asbp = ctx.enter_context(tc.tile_pool(name="asbp", bufs=4))

            # ---- constants -------------------------------------------------
            ident = singles.tile([128, 128], F32)
            make_identity(nc, ident[:])
            b_sb = singles.tile([128, 8], F32)  # conv biases, [ch-in-tile, oc-tile]
            nc.gpsimd.dma_start(out=b_sb, in_=ball_d.ap().rearrange("(t p) -> p t", p=128))
            bat_sb = singles.tile([128, 2], F32)
            nc.gpsimd.dma_start(out=bat_sb, in_=bat_d.ap().rearrange("(t p) -> p t", p=128))
            bv_sb = singles.tile([128, C], F32)  # v bias replicated across partitions
            nc.gpsimd.dma_start(
                out=bv_sb, in_=ball_d.ap()[3 * C:4 * C].partition_broadcast(128))
            zero1 = singles.tile([128, 1], F32)
            nc.vector.memset(zero1[:], 0.0)
            eshift_sb = singles.tile([128, 1], F32)
            nc.vector.memset(eshift_sb[:], ESHIFT)
            ones8 = singles.tile([128, 8, 1], F32)
            nc.vector.memset(ones8[:], 1.0)
            wup = singles.tile([128, 512], F16)  # PE warm-up fodder
            nc.vector.memset(wup[:], 0.0)

            # ---- load x (zero-padded by one column each side) and weights --
            x_sb = []
            for ct in range(NCT):
                t = xp.tile([128, W + 2], F16, tag="x", name=f"x{ct}")
                nc.vector.tensor_copy(t[:, 0:1], zero1[:])
                nc.vector.tensor_copy(t[:, W + 1:W + 2], zero1[:])
                nc.gpsimd.dma_start(out=t[:, 1:W + 1], in_=x_d.ap()[ct * 128:(ct + 1) * 128, :])
                x_sb.append(t)
            wt_sb = {}
            wtv_sb = {}
            for kk in range(3):
                for ct in range(NCT):
                    t = wtp.tile([128, OC], F16, tag="wt", name=f"wt{kk}_{ct}")
                    nc.gpsimd.dma_start(out=t, in_=wt_d.ap()[kk, ct * 128:(ct + 1) * 128, :])
                    wt_sb[kk, ct] = t
                    wtv_sb[kk, ct] = t[:, 3 * C:4 * C]
            watt_sb = []
            for ct in range(NCT):
                t = qkp.tile([128, C], F16, tag="watt", name=f"watt{ct}")
                nc.gpsimd.dma_start(out=t, in_=watt_d.ap()[ct * 128:(ct + 1) * 128, :])
                watt_sb.append(t)

            # ---- stage 1: q/k convs + vT conv ([ch,W] and [w,64*h] layouts) -
            q_sb, k_sb = [], []
            for qt in range(2):
                q_sb.append(qkp.tile([128, W], F16, tag="qk", name=f"q{qt}"))
                k_sb.append(qkp.tile([128, W], F16, tag="qk", name=f"k{qt}"))

            with tc.tile_pool(name="cps", bufs=4, space="PSUM") as cps:
                # PE warm-up burst: gets HAM to 8/8 while the input DMAs run
                wps = cps.tile([128, 512], F32, tag="cps", name="wps")
                for _ in range(26):
                    nc.tensor.matmul(wps[:], wup[:, 0:128], wup[:], start=True, stop=True)

                # vT conv: [w, 64*vh] layout; per head h a 64-wide block:
                # cols 0:32 v dims (+bias), col 32 ones (softmax denominator),
                # cols 33:64 zero.
                vt_sb = []
                for m in range(NMT):
                    vt = vtp.tile([128, NH * 64], F16, tag="vt", name=f"vt{m}")
                    nc.vector.memset(vt[:], 0.0)
                    nc.vector.tensor_copy(
                        vt[:].rearrange("p (h e) -> p h e", e=64)[:, :, 32:33],
                        ones8[:])
                    ps = cps.tile([128, C], F32, tag="vps")
                    for ct in range(NCT):
                        for kk in range(3):
                            nc.tensor.matmul(
                                ps[:],
                                x_sb[ct][:, m * 128 + kk:m * 128 + kk + 128],
                                wtv_sb[kk, ct],
                                start=(ct == 0 and kk == 0),
                                stop=(ct == NCT - 1 and kk == 2),
                            )
                    nc.vector.tensor_add(
                        vt[:].rearrange("p (h e) -> p h e", e=64)[:, :, 0:32],
                        ps[:].rearrange("p (h d) -> p h d", d=32),
                        bv_sb[:].rearrange("p (h d) -> p h d", d=32),
                    )
                    vt_sb.append(vt)

                for t in (2, 4):  # oc-tiles: q0, k0 (tile1 deferred)
                    for n in range(NWT):
                        ps = cps.tile([128, 512], F32, tag="cps")
                        for ct in range(NCT):
                            for kk in range(3):
                                nc.tensor.matmul(
                                    ps[:],
                                    wt_sb[kk, ct][:, t * 128:(t + 1) * 128],
                                    x_sb[ct][:, n * 512 + kk:n * 512 + kk + 512],
                                    start=(ct == 0 and kk == 0),
                                    stop=(ct == NCT - 1 and kk == 2),
                                )
                        ns = slice(n * 512, (n + 1) * 512)
                        if t < 4:        # q (scale folded; bias pre-scaled on host)
                            nc.vector.tensor_scalar(
                                out=q_sb[t - 2][:, ns], in0=ps[:],
                                scalar1=QSCALE, scalar2=b_sb[:, t:t + 1],
                                op0=mybir.AluOpType.mult, op1=mybir.AluOpType.add)
                        else:            # k
                            nc.vector.tensor_scalar_add(k_sb[t - 4][:, ns], ps[:], b_sb[:, t:t + 1])

            # ---- stage 2: attention, 4 heads concurrent ---------------------
            with tc.tile_pool(name="lg", bufs=2, space="PSUM") as lg, \
                 tc.tile_pool(name="aps", bufs=2, space="PSUM") as aps, \
                 tc.tile_pool(name="tps", bufs=2, space="PSUM") as tps:

                def conv_unit(t, n):
                    ps = tps.tile([128, 512], F32, tag="tps", name=f"cops{t}_{n}")
                    for ct in range(NCT):
                        for kk in range(3):
                            nc.tensor.matmul(
                                ps[:],
                                wt_sb[kk, ct][:, t * 128:(t + 1) * 128],
                                x_sb[ct][:, n * 512 + kk:n * 512 + kk + 512],
                                start=(ct == 0 and kk == 0),
                                stop=(ct == NCT - 1 and kk == 2),
                            )
                    ns = slice(n * 512, (n + 1) * 512)
                    if t < 2:      # conv_out -> DRAM
                        co = stage.tile([128, 512], F32, tag="co")
                        nc.vector.tensor_scalar_add(co[:], ps[:], b_sb[:, t:t + 1])
                        nc.sync.dma_start(out=out_d.ap()[t * 128:(t + 1) * 128, ns], in_=co[:])
                    elif t == 3:   # q tile 1
                        nc.vector.tensor_scalar(
                            out=q_sb[1][:, ns], in0=ps[:],
                            scalar1=QSCALE, scalar2=b_sb[:, t:t + 1],
                            op0=mybir.AluOpType.mult, op1=mybir.AluOpType.add)
                    else:          # k tile 1
                        nc.vector.tensor_scalar_add(k_sb[1][:, ns], ps[:], b_sb[:, t:t + 1])

                conv_units = ([(3, n) for n in range(NWT)]      # q tile 1 (for g=1)
                              + [(5, n) for n in range(NWT)]    # k tile 1
                              + [(t, n) for t in range(2) for n in range(NWT)])
                cu_i = 0

                # resident reshape source for stage 3: ar[c=(h,g), w2=(r,d)],
                # streamed back per finished head from the [h,w,d] DRAM staging
                ar_sb = [qkp.tile([128, W], F16, tag="ar", name=f"ar{i}") for i in range(2)]
                ar = ahwd_d.ap().rearrange("h (g x) d -> (h g) (x d)", g=32)  # [256, 2048]

                pending = []   # deferred normalize closures
                mstep = 0

                def pop_pending(k):
                    for _ in range(min(k, len(pending))):
                        pending.pop(0)()

                for g in range(2):
                    for n in range(NWT):
                        ns = slice(n * 512, (n + 1) * 512)
                        acc01 = aps.tile([128, 512], F32, tag="aps", name=f"a01_{g}_{n}")
                        acc23 = aps.tile([128, 512], F32, tag="aps", name=f"a23_{g}_{n}")
                        for m in range(NMT):
                            ms = slice(m * 128, (m + 1) * 128)
                            lg01 = lg.tile([128, 1024], F32, tag="lg", name=f"lg01_{g}_{n}_{m}")
                            lg23 = lg.tile([128, 1024], F32, tag="lg", name=f"lg23_{g}_{n}_{m}")
                            for hp in range(4):
                                dst = lg01 if hp < 2 else lg23
                                nc.tensor.matmul(
                                    dst[:, (hp % 2) * 512:(hp % 2) * 512 + 512],
                                    k_sb[g][32 * hp:32 * hp + 32, ms],
                                    q_sb[g][32 * hp:32 * hp + 32, ns],
                                    start=True, stop=True,
                                    tile_position=(32 * hp, 0))
                            e01 = ep.tile([128, 1024], F16, tag="e", name=f"e01_{g}_{n}_{m}")
                            e23 = ep.tile([128, 1024], F16, tag="e", name=f"e23_{g}_{n}_{m}")
                            nc.scalar.activation(e01[:], lg01[:],
                                                 mybir.ActivationFunctionType.Exp,
                                                 bias=eshift_sb[:])
                            if mstep % DVE_EXP_MOD != DVE_EXP_MOD - 1:
                                nc.vector.tensor_scalar(
                                    out=e23[:].bitcast(U16), in0=lg23[:],
                                    scalar1=SCHR_A, scalar2=SCHR_B,
                                    op0=mybir.AluOpType.mult,
                                    op1=mybir.AluOpType.add)
                            else:
                                nc.scalar.activation(e23[:], lg23[:],
                                                     mybir.ActivationFunctionType.Exp,
                                                     bias=eshift_sb[:])
                            for hp in range(4):
                                dst = acc01 if hp < 2 else acc23
                                src = e01 if hp < 2 else e23
                                nc.tensor.matmul(
                                    dst[64 * (hp % 2):64 * (hp % 2) + 64, :],
                                    vt_sb[m][:, 64 * (4 * g + hp):64 * (4 * g + hp) + 64],
                                    src[:, (hp % 2) * 512:(hp % 2) * 512 + 512],
                                    start=(m == 0), stop=(m == NMT - 1),
                                    tile_position=(0, 64 * (hp % 2)))
                            if mstep % CONV_EVERY == CONV_EVERY - 1 and cu_i < len(conv_units):
                                conv_unit(*conv_units[cu_i]); cu_i += 1
                            pop_pending(PENDING_PER_STEP)
                            mstep += 1

                        # free accumulators fast: one copy per acc pair, then
                        # defer transposes+normalize into the next block.
                        a01 = asbp.tile([128, 512], F32, tag="asb", name=f"as01_{g}_{n}")
                        a23 = asbp.tile([128, 512], F32, tag="asb", name=f"as23_{g}_{n}")
                        nc.vector.tensor_copy(a01[:], acc01[:])
                        nc.vector.tensor_copy(a23[:], acc23[:])

                        def make_norm(g, n, pair, a_sb, j):
                            # one transpose+normalize chunk: head-pair `pair`,
                            # q chunk j (128 wide) of this (g, n) block
                            def run():
                                for hh in range(2):
                                    h = 4 * g + 2 * pair + hh
                                    t_ps = tps.tile([128, 512], F32, tag="tps",
                                                    name=f"tp{h}_{n}_{j}")
                                    nc.tensor.matmul(
                                        t_ps[:, 0:33],
                                        a_sb[64 * hh:64 * hh + 33,
                                             j * 128:(j + 1) * 128],
                                        ident[64 * hh:64 * hh + 33,
                                              64 * hh:64 * hh + 33],
                                        start=True, stop=True,
                                        tile_position=(64 * hh, 0))
                                    r_sb = norm.tile([128, 1], F32, tag="r",
                                                     name=f"r{h}_{n}_{j}")
                                    nc.vector.reciprocal(r_sb[:], t_ps[:, 32:33])
                                    z_sb = norm.tile([128, 32], F16, tag="z",
                                                     name=f"z{h}_{n}_{j}")
                                    nc.vector.tensor_scalar_mul(z_sb[:], t_ps[:, 0:32], r_sb[:])
                                    ws = slice(n * 512 + j * 128, n * 512 + j * 128 + 128)
                                    nc.sync.dma_start(out=ahwd_d.ap()[h, ws, :], in_=z_sb[:])
                                    if n == NWT - 1 and j == 3:
                                        # head fully staged -> stream reshape rows in
                                        rr = slice((h % 4) * 32, (h % 4) * 32 + 32)
                                        nc.sync.dma_start(
                                            out=ar_sb[h // 4][rr, :],
                                            in_=ar[h * 32:(h + 1) * 32, :])
                            return run

                        for pair, a_sb in ((0, a01), (1, a23)):
                            for j in range(4):
                                pending.append(make_norm(g, n, pair, a_sb, j))

                while pending or cu_i < len(conv_units):
                    if cu_i < len(conv_units):
                        conv_unit(*conv_units[cu_i]); cu_i += 1
                    pop_pending(PENDING_PER_STEP)

                # ---- stage 3: 1x1 conv over the (faithful-reshape) view ----
                wk = lg.tile([128, 512], F32, tag="lg", name="warmkeep")
                for _ in range(12):
                    nc.tensor.matmul(wk[:], wup[:, 0:128], wup[:], start=True, stop=True)
                for t2 in range(2):
                    for n in range(NWT):
                        ps = lg.tile([128, 512], F32, tag="lg", name=f"fin{t2}_{n}")
                        for ct in range(NCT):
                            nc.tensor.matmul(
                                ps[:], watt_sb[ct][:, t2 * 128:(t2 + 1) * 128],
                                ar_sb[ct][:, n * 512:(n + 1) * 512],
                                start=(ct == 0), stop=(ct == NCT - 1))
                        fo = stage.tile([128, 512], F32, tag="fo")
                        nc.vector.tensor_scalar_add(fo[:], ps[:], bat_sb[:, t2:t2 + 1])
                        nc.sync.dma_start(
                            out=out_d.ap()[C + t2 * 128:C + (t2 + 1) * 128,
                                           n * 512:(n + 1) * 512],
                            in_=fo[:])

    nc.compile()
    return nc


_NC_CACHE = []


def _get_nc():
    if not _NC_CACHE:
        _NC_CACHE.append(build_nc())
    return _NC_CACHE[0]


def _prep_in_maps(x, w_conv, b_conv, w_qkv, b_qkv, w_attn, b_attn):
    x = np.asarray(x, np.float16)
    wt = np.ascontiguousarray(
        np.concatenate([np.asarray(w_conv, np.float32), np.asarray(w_qkv, np.float32)], 0)
        .transpose(2, 1, 0).astype(np.float16))                # [3, c, oc]
    b_all = np.concatenate([np.asarray(b_conv, np.float32),
                            np.asarray(b_qkv, np.float32)]).copy()
    b_all[C:2 * C] *= QSCALE                                   # q bias pre-scaled
    watt = np.ascontiguousarray(np.asarray(w_attn, np.float32).T.astype(np.float16))
    bat = np.ascontiguousarray(np.asarray(b_attn, np.float32))
    return [
        {"x": np.ascontiguousarray(x[b]), "wt": wt, "b_all": b_all,
         "watT": watt, "bat": bat}
        for b in range(x.shape[0])
    ]


def run(trace=False, **inputs):
    nc = _get_nc()
    in_maps = _prep_in_maps(**inputs)
    res = bass_utils.run_bass_kernel_spmd(
        nc, in_maps, core_ids=list(range(8)), trace=trace,
        **({"trace_cores": [0]} if trace else {}))
    out = np.stack([res.results[i]["out"] for i in range(8)]).astype(np.float32)
    return out, res


def kernel(**inputs) -> np.ndarray:
    out, _ = run(**inputs)
    return out


# revision 13
# speedup vs baseline: 1.1226x; 1.1226x over previous
"""Trainium2 Bass kernel for nn_AugmentedConv (conv branch + conv-attention branch).

Full-input contract: kernel(**inputs) takes the complete unsharded inputs and
returns the full (8, 512, 2048) output. Internally: data-parallel over batch
across 8 NeuronCores; each core runs the whole module for one batch element.

v2: PE tile_position packing + split exp across ScalarE/DVE.
 - logits: 4 heads concurrent via 32-row tile_position groups (contract=32).
 - attnV: 2 heads per PSUM bank via 64-col tile_position groups; each head
   block in vt is 64 wide (32 v dims + ones col for the softmax denominator).
 - exp: ScalarE exact exp for half the tiles; DVE Schraudolph uint16 bit-trick
   (out = bits(round(x*a+b)) viewed as fp16, saturating: negatives -> +0) for
   the rest. ~3% per-element error on those tiles; cancels largely in softmax.

Hardcoded problem shapes: B=8, C=256, W=2048, DK=DV=256, NH=8, KS=3, pad=1.
"""

import numpy as np

import concourse.bacc as bacc
import concourse.mybir as mybir
import concourse.tile as tile
from concourse import bass_utils
from concourse.masks import make_identity

F32 = mybir.dt.float32
F16 = mybir.dt.float16
BF16 = mybir.dt.bfloat16
U16 = mybir.dt.uint16
ESHIFT = -4.0   # exp(x + ESHIFT): keeps fp16 exp in range; cancels in softmax ratio

C = 256          # input channels
W = 2048         # sequence length
OC = 1024        # combined conv output channels: [conv_out 256 | q 256 | k 256 | v 256]
NH = 8
DKH = 32         # head dim (dk and dv per head)
QSCALE = float(DKH) ** -0.5
NCT = C // 128   # input-channel tiles (2)
NWT = W // 512   # 512-wide w tiles (4)
NMT = W // 128   # 128-wide w tiles (16)

LOG2E = 1.4426950408889634
SCHR_A = 1024.0 * LOG2E
SCHR_B = ESHIFT * SCHR_A + 15 * 1024 - 46.0

# knobs
DVE_EXP_MOD = 3      # DVE handles lg23 when (mstep % DVE_EXP_MOD) != DVE_EXP_MOD-1
CONV_EVERY = 7       # emit one conv unit every N m-steps
PENDING_PER_STEP = 3 # deferred normalize items popped per m-step


def build_nc():
    nc = bacc.Bacc("TRN2", target_bir_lowering=False, debug=False)

    x_d = nc.dram_tensor("x", [C, W], F16, kind="ExternalInput")
    wt_d = nc.dram_tensor("wt", [3, C, OC], F16, kind="ExternalInput")     # wt[k,c,oc]
    ball_d = nc.dram_tensor("b_all", [OC], F32, kind="ExternalInput")      # q part pre-scaled
    watt_d = nc.dram_tensor("watT", [C, C], F16, kind="ExternalInput")     # w_attn.T (c,o)
    bat_d = nc.dram_tensor("bat", [C], F32, kind="ExternalInput")
    out_d = nc.dram_tensor("out", [2 * C, W], F32, kind="ExternalOutput")
    # attention output staged in [h, w, d] order; the module's faithful
    # (NH,W,dvh)->(256,W) reshape is then a contiguous view of this buffer.
    ahwd_d = nc.dram_tensor("attn_hwd", [NH, W, DKH], F16)

    with tile.TileContext(nc) as tc:
        import contextlib
        with contextlib.ExitStack() as ctx:
            singles = ctx.enter_context(tc.tile_pool(name="singles", bufs=1))
            xp = ctx.enter_context(tc.tile_pool(name="xp", bufs=NCT))
            wtp = ctx.enter_context(tc.tile_pool(name="wtp", bufs=3 * NCT))
            qkp = ctx.enter_context(tc.tile_pool(name="qkp", bufs=4))
            vtp = ctx.enter_context(tc.tile_pool(name="vtp", bufs=NMT))
            ep = ctx.enter_context(tc.tile_pool(name="ep", bufs=6))
            stage = ctx.enter_context(tc.tile_pool(name="stage", bufs=4))
            norm = ctx.enter_context(tc.tile_pool(name="norm", bufs=10))
            asbp = ctx.enter_context(tc.tile_pool(name="asbp", bufs=4))

            # ---- constants -------------------------------------------------
            ident = singles.tile([128, 128], F32)
            make_identity(nc, ident[:])
            identb = singles.tile([128, 128], BF16)
            nc.vector.tensor_copy(identb[:], ident[:])
            b_sb = singles.tile([128, 8], F32)  # conv biases, [ch-in-tile, oc-tile]
            nc.gpsimd.dma_start(out=b_sb, in_=ball_d.ap().rearrange("(t p) -> p t", p=128))
            bat_sb = singles.tile([128, 2], F32)
            nc.gpsimd.dma_start(out=bat_sb, in_=bat_d.ap().rearrange("(t p) -> p t", p=128))
            bv_sb = singles.tile([128, C], F32)  # v bias replicated across partitions
            nc.gpsimd.dma_start(
                out=bv_sb, in_=ball_d.ap()[3 * C:4 * C].partition_broadcast(128))
            zero1 = singles.tile([128, 1], F32)
            nc.vector.memset(zero1[:], 0.0)
            eshift_sb = singles.tile([128, 1], F32)
            nc.vector.memset(eshift_sb[:], ESHIFT)
            ones8 = singles.tile([128, 8, 1], F32)
            nc.vector.memset(ones8[:], 1.0)
            wup = singles.tile([128, 512], F16)  # PE warm-up fodder
            nc.vector.memset(wup[:], 0.0)

            # ---- load x (zero-padded by one column each side) and weights --
            x_sb = []
            for ct in range(NCT):
                t = xp.tile([128, W + 2], F16, tag="x", name=f"x{ct}")
                nc.vector.tensor_copy(t[:, 0:1], zero1[:])
                nc.vector.tensor_copy(t[:, W + 1:W + 2], zero1[:])
                nc.gpsimd.dma_start(out=t[:, 1:W + 1], in_=x_d.ap()[ct * 128:(ct + 1) * 128, :])
                x_sb.append(t)
            wt_sb = {}
            wtv_sb = {}
            for kk in range(3):
                for ct in range(NCT):
                    t = wtp.tile([128, OC], F16, tag="wt", name=f"wt{kk}_{ct}")
                    nc.gpsimd.dma_start(out=t, in_=wt_d.ap()[kk, ct * 128:(ct + 1) * 128, :])
                    wt_sb[kk, ct] = t
                    wtv_sb[kk, ct] = t[:, 3 * C:4 * C]
            watt_sb = []
            for ct in range(NCT):
                t = qkp.tile([128, C], F16, tag="watt", name=f"watt{ct}")
                nc.gpsimd.dma_start(out=t, in_=watt_d.ap()[ct * 128:(ct + 1) * 128, :])
                watt_sb.append(t)

            # ---- stage 1: q/k convs + vT conv ([ch,W] and [w,64*h] layouts) -
            q_sb, k_sb = [], []
            for qt in range(2):
                q_sb.append(qkp.tile([128, W], F16, tag="qk", name=f"q{qt}"))
                k_sb.append(qkp.tile([128, W], F16, tag="qk", name=f"k{qt}"))

            with tc.tile_pool(name="cps", bufs=4, space="PSUM") as cps:
                # PE warm-up burst: gets HAM to 8/8 while the input DMAs run
                wps = cps.tile([128, 512], F32, tag="cps", name="wps")
                for _ in range(16):
                    nc.tensor.matmul(wps[:], wup[:, 0:128], wup[:], start=True, stop=True)

                # vT conv: [w, 64*vh] layout; per head h a 64-wide block:
                # cols 0:32 v dims (+bias), col 32 ones (softmax denominator),
                # cols 33:64 zero.
                vt_sb = []
                for m in range(NMT):
                    vt = vtp.tile([128, NH * 64], F16, tag="vt", name=f"vt{m}")
                    nc.vector.memset(vt[:], 0.0)
                    nc.vector.tensor_copy(
                        vt[:].rearrange("p (h e) -> p h e", e=64)[:, :, 32:33],
                        ones8[:])
                    ps = cps.tile([128, C], F32, tag="vps")
                    for ct in range(NCT):
                        for kk in range(3):
                            nc.tensor.matmul(
                                ps[:],
                                x_sb[ct][:, m * 128 + kk:m * 128 + kk + 128],
                                wtv_sb[kk, ct],
                                start=(ct == 0 and kk == 0),
                                stop=(ct == NCT - 1 and kk == 2),
                            )
                    nc.vector.tensor_add(
                        vt[:].rearrange("p (h e) -> p h e", e=64)[:, :, 0:32],
                        ps[:].rearrange("p (h d) -> p h d", d=32),
                        bv_sb[:].rearrange("p (h d) -> p h d", d=32),
                    )
                    vt_sb.append(vt)

                for t in (2, 4):  # oc-tiles: q0, k0 (tile1 deferred)
                    for n in range(NWT):
                        ps = cps.tile([128, 512], F32, tag="cps")
                        for ct in range(NCT):
                            for kk in range(3):
                                nc.tensor.matmul(
                                    ps[:],
                                    wt_sb[kk, ct][:, t * 128:(t + 1) * 128],
                                    x_sb[ct][:, n * 512 + kk:n * 512 + kk + 512],
                                    start=(ct == 0 and kk == 0),
                                    stop=(ct == NCT - 1 and kk == 2),
                                )
                        ns = slice(n * 512, (n + 1) * 512)
                        if t < 4:        # q (scale folded; bias pre-scaled on host)
                            nc.vector.tensor_scalar(
                                out=q_sb[t - 2][:, ns], in0=ps[:],
                                scalar1=QSCALE, scalar2=b_sb[:, t:t + 1],
                                op0=mybir.AluOpType.mult, op1=mybir.AluOpType.add)
                        else:            # k
                            nc.vector.tensor_scalar_add(k_sb[t - 4][:, ns], ps[:], b_sb[:, t:t + 1])

            # ---- stage 2: attention, 4 heads concurrent ---------------------
            with tc.tile_pool(name="lg", bufs=2, space="PSUM") as lg, \
                 tc.tile_pool(name="aps", bufs=2, space="PSUM") as aps, \
                 tc.tile_pool(name="tps", bufs=2, space="PSUM") as tps:

                def conv_unit(t, n):
                    ps = tps.tile([128, 512], F32, tag="tps", name=f"cops{t}_{n}")
                    for ct in range(NCT):
                        for kk in range(3):
                            nc.tensor.matmul(
                                ps[:],
                                wt_sb[kk, ct][:, t * 128:(t + 1) * 128],
                                x_sb[ct][:, n * 512 + kk:n * 512 + kk + 512],
                                start=(ct == 0 and kk == 0),
                                stop=(ct == NCT - 1 and kk == 2),
                            )
                    ns = slice(n * 512, (n + 1) * 512)
                    if t < 2:      # conv_out -> DRAM
                        co = stage.tile([128, 512], F32, tag="co")
                        nc.vector.tensor_scalar_add(co[:], ps[:], b_sb[:, t:t + 1])
                        nc.sync.dma_start(out=out_d.ap()[t * 128:(t + 1) * 128, ns], in_=co[:])
                    elif t == 3:   # q tile 1
                        nc.vector.tensor_scalar(
                            out=q_sb[1][:, ns], in0=ps[:],
                            scalar1=QSCALE, scalar2=b_sb[:, t:t + 1],
                            op0=mybir.AluOpType.mult, op1=mybir.AluOpType.add)
                    else:          # k tile 1
                        nc.vector.tensor_scalar_add(k_sb[1][:, ns], ps[:], b_sb[:, t:t + 1])

                conv_units = ([(3, n) for n in range(NWT)]      # q tile 1 (for g=1)
                              + [(5, n) for n in range(NWT)]    # k tile 1
                              + [(t, n) for t in range(2) for n in range(NWT)])
                cu_i = 0

                # resident reshape source for stage 3: ar[c=(h,g), w2=(r,d)],
                # streamed back per finished head from the [h,w,d] DRAM staging
                ar_sb = [qkp.tile([128, W], F16, tag="ar", name=f"ar{i}") for i in range(2)]
                ar = ahwd_d.ap().rearrange("h (g x) d -> (h g) (x d)", g=32)  # [256, 2048]

                pending = []   # deferred normalize closures
                mstep = 0

                def pop_pending(k):
                    for _ in range(min(k, len(pending))):
                        pending.pop(0)()

                for g in range(2):
                    for n in range(NWT):
                        ns = slice(n * 512, (n + 1) * 512)
                        acc01 = aps.tile([128, 512], F32, tag="aps", name=f"a01_{g}_{n}")
                        acc23 = aps.tile([128, 512], F32, tag="aps", name=f"a23_{g}_{n}")
                        for m in range(NMT):
                            ms = slice(m * 128, (m + 1) * 128)
                            lg01 = lg.tile([128, 1024], F32, tag="lg", name=f"lg01_{g}_{n}_{m}")
                            lg23 = lg.tile([128, 1024], F32, tag="lg", name=f"lg23_{g}_{n}_{m}")
                            for hp in range(4):
                                dst = lg01 if hp < 2 else lg23
                                nc.tensor.matmul(
                                    dst[:, (hp % 2) * 512:(hp % 2) * 512 + 512],
                                    k_sb[g][32 * hp:32 * hp + 32, ms],
                                    q_sb[g][32 * hp:32 * hp + 32, ns],
                                    start=True, stop=True,
                                    tile_position=(32 * hp, 0))
                            e01 = ep.tile([128, 1024], F16, tag="e", name=f"e01_{g}_{n}_{m}")
                            e23 = ep.tile([128, 1024], F16, tag="e", name=f"e23_{g}_{n}_{m}")
                            nc.scalar.activation(e01[:], lg01[:],
                                                 mybir.ActivationFunctionType.Exp,
                                                 bias=eshift_sb[:])
                            if mstep % DVE_EXP_MOD != DVE_EXP_MOD - 1:
                                nc.vector.tensor_scalar(
                                    out=e23[:].bitcast(U16), in0=lg23[:],
                                    scalar1=SCHR_A, scalar2=SCHR_B,
                                    op0=mybir.AluOpType.mult,
                                    op1=mybir.AluOpType.add)
                            else:
                                nc.scalar.activation(e23[:], lg23[:],
                                                     mybir.ActivationFunctionType.Exp,
                                                     bias=eshift_sb[:])
                            for hp in range(4):
                                dst = acc01 if hp < 2 else acc23
                                src = e01 if hp < 2 else e23
                                nc.tensor.matmul(
                                    dst[64 * (hp % 2):64 * (hp % 2) + 64, :],
                                    vt_sb[m][:, 64 * (4 * g + hp):64 * (4 * g + hp) + 64],
                                    src[:, (hp % 2) * 512:(hp % 2) * 512 + 512],
                                    start=(m == 0), stop=(m == NMT - 1),
                                    tile_position=(0, 64 * (hp % 2)))
                            if mstep % CONV_EVERY == CONV_EVERY - 1 and cu_i < len(conv_units):
                                conv_unit(*conv_units[cu_i]); cu_i += 1
                            pop_pending(PENDING_PER_STEP)
                            mstep += 1

                        # free accumulators fast: one copy per acc pair, then
                        # defer transposes+normalize into the next block.
                        a01 = asbp.tile([128, 512], BF16, tag="asb", name=f"as01_{g}_{n}")
                        a23 = asbp.tile([128, 512], BF16, tag="asb", name=f"as23_{g}_{n}")
                        nc.vector.tensor_copy(a01[:], acc01[:])
                        nc.vector.tensor_copy(a23[:], acc23[:])

                        def make_norm(g, n, pair, a_sb, j):
                            # one transpose+normalize chunk: head-pair `pair`,
                            # q chunk j (128 wide) of this (g, n) block
                            def run():
                                for hh in range(2):
                                    h = 4 * g + 2 * pair + hh
                                    t_ps = tps.tile([128, 512], F32, tag="tps",
                                                    name=f"tp{h}_{n}_{j}")
                                    nc.tensor.matmul(
                                        t_ps[:, 0:33],
                                        a_sb[64 * hh:64 * hh + 33,
                                             j * 128:(j + 1) * 128],
                                        identb[64 * hh:64 * hh + 33,
                                               64 * hh:64 * hh + 33],
                                        start=True, stop=True,
                                        tile_position=(64 * hh, 0))
                                    r_sb = norm.tile([128, 1], F32, tag="r",
                                                     name=f"r{h}_{n}_{j}")
                                    nc.vector.reciprocal(r_sb[:], t_ps[:, 32:33])
                                    z_sb = norm.tile([128, 32], F16, tag="z",
                                                     name=f"z{h}_{n}_{j}")
                                    nc.vector.tensor_scalar_mul(z_sb[:], t_ps[:, 0:32], r_sb[:])
                                    ws = slice(n * 512 + j * 128, n * 512 + j * 128 + 128)
                                    nc.sync.dma_start(out=ahwd_d.ap()[h, ws, :], in_=z_sb[:])
                                    if n == NWT - 1 and j == 3:
                                        # head fully staged -> stream reshape rows in
                                        rr = slice((h % 4) * 32, (h % 4) * 32 + 32)
                                        nc.sync.dma_start(
                                            out=ar_sb[h // 4][rr, :],
                                            in_=ar[h * 32:(h + 1) * 32, :])
                            return run

                        for pair, a_sb in ((0, a01), (1, a23)):
                            for j in range(4):
                                pending.append(make_norm(g, n, pair, a_sb, j))

                while pending or cu_i < len(conv_units):
                    if cu_i < len(conv_units):
                        conv_unit(*conv_units[cu_i]); cu_i += 1
                    pop_pending(PENDING_PER_STEP)

                # ---- stage 3: 1x1 conv over the (faithful-reshape) view ----
                wk = lg.tile([128, 512], F32, tag="lg", name="warmkeep")
                for _ in range(6):
                    nc.tensor.matmul(wk[:], wup[:, 0:128], wup[:], start=True, stop=True)
                for t2 in range(2):
                    for n in range(NWT):
                        ps = lg.tile([128, 512], F32, tag="lg", name=f"fin{t2}_{n}")
                        for ct in range(NCT):
                            nc.tensor.matmul(
                                ps[:], watt_sb[ct][:, t2 * 128:(t2 + 1) * 128],
                                ar_sb[ct][:, n * 512:(n + 1) * 512],
                                start=(ct == 0), stop=(ct == NCT - 1))
                        fo = stage.tile([128, 512], F32, tag="fo")
                        nc.vector.tensor_scalar_add(fo[:], ps[:], bat_sb[:, t2:t2 + 1])
                        nc.sync.dma_start(
                            out=out_d.ap()[C + t2 * 128:C + (t2 + 1) * 128,
                                           n * 512:(n + 1) * 512],
                            in_=fo[:])

    nc.compile()
    return nc


_NC_CACHE = []


def _get_nc():
    if not _NC_CACHE:
        _NC_CACHE.append(build_nc())
    return _NC_CACHE[0]


def _prep_in_maps(x, w_conv, b_conv, w_qkv, b_qkv, w_attn, b_attn):
    x = np.asarray(x, np.float16)
    wt = np.ascontiguousarray(
        np.concatenate([np.asarray(w_conv, np.float32), np.asarray(w_qkv, np.float32)], 0)
        .transpose(2, 1, 0).astype(np.float16))                # [3, c, oc]
    b_all = np.concatenate([np.asarray(b_conv, np.float32),
                            np.asarray(b_qkv, np.float32)]).copy()
    b_all[C:2 * C] *= QSCALE                                   # q bias pre-scaled
    watt = np.ascontiguousarray(np.asarray(w_attn, np.float32).T.astype(np.float16))
    bat = np.ascontiguousarray(np.asarray(b_attn, np.float32))
    return [
        {"x": np.ascontiguousarray(x[b]), "wt": wt, "b_all": b_all,
         "watT": watt, "bat": bat}
        for b in range(x.shape[0])
    ]


def run(trace=False, **inputs):
    nc = _get_nc()
    in_maps = _prep_in_maps(**inputs)
    res = bass_utils.run_bass_kernel_spmd(
        nc, in_maps, core_ids=list(range(8)), trace=trace,
        **({"trace_cores": [0]} if trace else {}))
    out = np.stack([res.results[i]["out"] for i in range(8)]).astype(np.float32)
    return out, res


def kernel(**inputs) -> np.ndarray:
    out, _ = run(**inputs)
    return out


# revision 18
# speedup vs baseline: 1.1777x; 1.0491x over previous
"""Trainium2 Bass kernel for nn_AugmentedConv (conv branch + conv-attention branch).

Full-input contract: kernel(**inputs) takes the complete unsharded inputs and
returns the full (8, 512, 2048) output. Internally: data-parallel over batch
across 8 NeuronCores; each core runs the whole module for one batch element.

v2: PE tile_position packing + split exp across ScalarE/DVE.
 - logits: 4 heads concurrent via 32-row tile_position groups (contract=32).
 - attnV: 2 heads per PSUM bank via 64-col tile_position groups; each head
   block in vt is 64 wide (32 v dims + ones col for the softmax denominator).
 - exp: ScalarE exact exp for half the tiles; DVE Schraudolph uint16 bit-trick
   (out = bits(round(x*a+b)) viewed as fp16, saturating: negatives -> +0) for
   the rest. ~3% per-element error on those tiles; cancels largely in softmax.

Hardcoded problem shapes: B=8, C=256, W=2048, DK=DV=256, NH=8, KS=3, pad=1.
"""

import numpy as np

import concourse.bacc as bacc
import concourse.mybir as mybir
import concourse.tile as tile
from concourse import bass_utils
from concourse.masks import make_identity

F32 = mybir.dt.float32
F16 = mybir.dt.float16
BF16 = mybir.dt.bfloat16
U16 = mybir.dt.uint16
ESHIFT = -4.0   # exp(x + ESHIFT): keeps fp16 exp in range; cancels in softmax ratio

C = 256          # input channels
W = 2048         # sequence length
OC = 1024        # combined conv output channels: [conv_out 256 | q 256 | k 256 | v 256]
NH = 8
DKH = 32         # head dim (dk and dv per head)
QSCALE = float(DKH) ** -0.5
NCT = C // 128   # input-channel tiles (2)
NWT = W // 512   # 512-wide w tiles (4)
NMT = W // 128   # 128-wide w tiles (16)

LOG2E = 1.4426950408889634
SCHR_A = 1024.0 * LOG2E
SCHR_B = ESHIFT * SCHR_A + 15 * 1024 - 46.0

# knobs
CONV_EVERY = 14      # emit one conv unit every N m-steps
PENDING_PER_STEP = 3 # deferred normalize items popped per m-step


def build_nc():
    nc = bacc.Bacc("TRN2", target_bir_lowering=False, debug=False)

    x_d = nc.dram_tensor("x", [C, W], F16, kind="ExternalInput")
    wt_d = nc.dram_tensor("wt", [3, C, OC], F16, kind="ExternalInput")     # wt[k,c,oc]
    ball_d = nc.dram_tensor("b_all", [OC], F32, kind="ExternalInput")      # q part pre-scaled
    watt_d = nc.dram_tensor("watT", [C, C], F16, kind="ExternalInput")     # w_attn.T (c,o)
    bat_d = nc.dram_tensor("bat", [C], F32, kind="ExternalInput")
    out_d = nc.dram_tensor("out", [2 * C, W], F32, kind="ExternalOutput")
    # attention output staged in [h, w, d] order; the module's faithful
    # (NH,W,dvh)->(256,W) reshape is then a contiguous view of this buffer.
    ahwd_d = nc.dram_tensor("attn_hwd", [NH, W, DKH], F16)

    with tile.TileContext(nc) as tc:
        import contextlib
        with contextlib.ExitStack() as ctx:
            singles = ctx.enter_context(tc.tile_pool(name="singles", bufs=1))
            xp = ctx.enter_context(tc.tile_pool(name="xp", bufs=NCT))
            wtp = ctx.enter_context(tc.tile_pool(name="wtp", bufs=3 * NCT))
            qkp = ctx.enter_context(tc.tile_pool(name="qkp", bufs=4))
            vtp = ctx.enter_context(tc.tile_pool(name="vtp", bufs=NMT))
            ep = ctx.enter_context(tc.tile_pool(name="ep", bufs=6))
            stage = ctx.enter_context(tc.tile_pool(name="stage", bufs=4))
            norm = ctx.enter_context(tc.tile_pool(name="norm", bufs=10))
            asbp = ctx.enter_context(tc.tile_pool(name="asbp", bufs=4))
            qpp = ctx.enter_context(tc.tile_pool(name="qpp", bufs=4))

            # ---- constants -------------------------------------------------
            ident = singles.tile([128, 128], F32)
            make_identity(nc, ident[:])
            identb = singles.tile([128, 128], BF16)
            nc.vector.tensor_copy(identb[:], ident[:])
            b_sb = singles.tile([128, 8], F32)  # conv biases, [ch-in-tile, oc-tile]
            nc.gpsimd.dma_start(out=b_sb, in_=ball_d.ap().rearrange("(t p) -> p t", p=128))
            bat_sb = singles.tile([128, 2], F32)
            nc.gpsimd.dma_start(out=bat_sb, in_=bat_d.ap().rearrange("(t p) -> p t", p=128))
            bv_sb = singles.tile([128, C], F32)  # v bias replicated across partitions
            nc.gpsimd.dma_start(
                out=bv_sb, in_=ball_d.ap()[3 * C:4 * C].partition_broadcast(128))
            zero1 = singles.tile([128, 1], F32)
            nc.vector.memset(zero1[:], 0.0)
            eshift_sb = singles.tile([128, 1], F32)
            nc.vector.memset(eshift_sb[:], ESHIFT)
            ones8 = singles.tile([128, 8, 1], F32)
            nc.vector.memset(ones8[:], 1.0)
            wup = singles.tile([128, 512], F16)  # PE warm-up fodder
            nc.vector.memset(wup[:], 0.0)

            # ---- load x (zero-padded by one column each side) and weights --
            x_sb = []
            for ct in range(NCT):
                t = xp.tile([128, W + 2], F16, tag="x", name=f"x{ct}")
                nc.vector.tensor_copy(t[:, 0:1], zero1[:])
                nc.vector.tensor_copy(t[:, W + 1:W + 2], zero1[:])
                nc.gpsimd.dma_start(out=t[:, 1:W + 1], in_=x_d.ap()[ct * 128:(ct + 1) * 128, :])
                x_sb.append(t)
            wt_sb = {}
            wtv_sb = {}
            for kk in range(3):
                for ct in range(NCT):
                    t = wtp.tile([128, OC], F16, tag="wt", name=f"wt{kk}_{ct}")
                    nc.gpsimd.dma_start(out=t, in_=wt_d.ap()[kk, ct * 128:(ct + 1) * 128, :])
                    wt_sb[kk, ct] = t
                    wtv_sb[kk, ct] = t[:, 3 * C:4 * C]
            watt_sb = []
            for ct in range(NCT):
                t = qkp.tile([128, C], F16, tag="watt", name=f"watt{ct}")
                nc.gpsimd.dma_start(out=t, in_=watt_d.ap()[ct * 128:(ct + 1) * 128, :])
                watt_sb.append(t)

            # ---- stage 1: q/k convs + vT conv ([ch,W] and [w,64*h] layouts) -
            q_sb, k_sb = [], []
            for qt in range(2):
                q_sb.append(qkp.tile([128, W], F16, tag="qk", name=f"q{qt}"))
                k_sb.append(qkp.tile([128, W], F16, tag="qk", name=f"k{qt}"))

            with tc.tile_pool(name="cps", bufs=4, space="PSUM") as cps:
                # PE warm-up burst: gets HAM to 8/8 while the input DMAs run
                wps = cps.tile([128, 512], F32, tag="cps", name="wps")
                for _ in range(16):
                    nc.tensor.matmul(wps[:], wup[:, 0:128], wup[:], start=True, stop=True)

                # vT conv: [w, vch] layout (x slice is the stationary operand),
                # v bias added here; col 32 of each 33-wide head block stays 1.0
                # (softmax-denominator ones column), cols >=264 zero pad so the
                # 128-col lhsT window at h=7 stays in range.
                vt_sb = []
                for m in range(NMT):
                    vt = vtp.tile([128, NH * 33 + 96], F16, tag="vt", name=f"vt{m}")
                    nc.vector.tensor_copy(
                        vt[:, 0:NH * 33].rearrange("p (h e) -> p h e", e=33)[:, :, 32:33],
                        ones8[:])
                    nc.vector.memset(vt[:, NH * 33:], 0.0)
                    ps = cps.tile([128, C], F32, tag="vps")
                    for ct in range(NCT):
                        for kk in range(3):
                            nc.tensor.matmul(
                                ps[:],
                                x_sb[ct][:, m * 128 + kk:m * 128 + kk + 128],
                                wtv_sb[kk, ct],
                                start=(ct == 0 and kk == 0),
                                stop=(ct == NCT - 1 and kk == 2),
                            )
                    nc.vector.tensor_add(
                        vt[:, 0:NH * 33].rearrange("p (h e) -> p h e", e=33)[:, :, 0:32],
                        ps[:].rearrange("p (h d) -> p h d", d=32),
                        bv_sb[:].rearrange("p (h d) -> p h d", d=32),
                    )
                    vt_sb.append(vt)

                for t in (2, 4):  # oc-tiles: q0, k0 (tile1 deferred)
                    for n in range(NWT):
                        ps = cps.tile([128, 512], F32, tag="cps")
                        for ct in range(NCT):
                            for kk in range(3):
                                nc.tensor.matmul(
                                    ps[:],
                                    wt_sb[kk, ct][:, t * 128:(t + 1) * 128],
                                    x_sb[ct][:, n * 512 + kk:n * 512 + kk + 512],
                                    start=(ct == 0 and kk == 0),
                                    stop=(ct == NCT - 1 and kk == 2),
                                )
                        ns = slice(n * 512, (n + 1) * 512)
                        if t < 4:        # q (scale folded; bias pre-scaled on host)
                            nc.vector.tensor_scalar(
                                out=q_sb[t - 2][:, ns], in0=ps[:],
                                scalar1=QSCALE, scalar2=b_sb[:, t:t + 1],
                                op0=mybir.AluOpType.mult, op1=mybir.AluOpType.add)
                        else:            # k
                            nc.vector.tensor_scalar_add(k_sb[t - 4][:, ns], ps[:], b_sb[:, t:t + 1])

            # ---- stage 2: attention, 2 heads per m-step -------------------
            # Per m-step: one [128,1024] logits pair tile (two FWL-friendly
            # full-contract matmuls against a zero-padded per-head q), one exp
            # (ScalarE exact or DVE Schraudolph by pattern), two attnV matmuls
            # (33-wide vt blocks via 128-col windows, ones col = denominator).
            def build_qpad(h):
                qt = h // 4
                s = 32 * (h % 4)
                qpad = qpp.tile([128, W], F16, tag="qpad", name=f"qpad{h}")
                nc.vector.memset(qpad[:], 0.0)
                nc.vector.tensor_copy(qpad[s:s + 32, :], q_sb[qt][s:s + 32, :])
                return qpad

            qpads = {0: build_qpad(0), 1: build_qpad(1)}

            with tc.tile_pool(name="lg", bufs=2, space="PSUM") as lg, \
                 tc.tile_pool(name="aps", bufs=2, space="PSUM") as aps, \
                 tc.tile_pool(name="tps", bufs=2, space="PSUM") as tps:

                def conv_unit(t, n):
                    ps = tps.tile([128, 512], F32, tag="tps", name=f"cops{t}_{n}")
                    for ct in range(NCT):
                        for kk in range(3):
                            nc.tensor.matmul(
                                ps[:],
                                wt_sb[kk, ct][:, t * 128:(t + 1) * 128],
                                x_sb[ct][:, n * 512 + kk:n * 512 + kk + 512],
                                start=(ct == 0 and kk == 0),
                                stop=(ct == NCT - 1 and kk == 2),
                            )
                    ns = slice(n * 512, (n + 1) * 512)
                    if t < 2:      # conv_out -> DRAM
                        co = stage.tile([128, 512], F32, tag="co")
                        nc.vector.tensor_scalar_add(co[:], ps[:], b_sb[:, t:t + 1])
                        nc.sync.dma_start(out=out_d.ap()[t * 128:(t + 1) * 128, ns], in_=co[:])
                    elif t == 3:   # q tile 1
                        nc.vector.tensor_scalar(
                            out=q_sb[1][:, ns], in0=ps[:],
                            scalar1=QSCALE, scalar2=b_sb[:, t:t + 1],
                            op0=mybir.AluOpType.mult, op1=mybir.AluOpType.add)
                    else:          # k tile 1
                        nc.vector.tensor_scalar_add(k_sb[1][:, ns], ps[:], b_sb[:, t:t + 1])

                # q tile 1 first: build_qpad(4/5) (emitted in g2=1) reads it,
                # and program order is semantic for those stale-read hazards.
                conv_units = ([(3, n) for n in range(NWT)]
                              + [(5, n) for n in range(NWT)]
                              + [(t, n) for t in range(2) for n in range(NWT)])
                cu_i = 0

                # resident reshape source for stage 3: ar[c=(h,g), w2=(r,d)],
                # streamed back per finished head from the [h,w,d] DRAM staging
                ar_sb = [qkp.tile([128, W], F16, tag="ar", name=f"ar{i}") for i in range(2)]
                ar = ahwd_d.ap().rearrange("h (g x) d -> (h g) (x d)", g=32)  # [256, 2048]

                pending = []   # deferred normalize closures
                mstep = 0

                def pop_pending(k):
                    for _ in range(min(k, len(pending))):
                        pending.pop(0)()

                def make_norm(h, n, a_sb, j):
                    # one transpose+normalize chunk: head h, q chunk j
                    def run():
                        t_ps = tps.tile([128, 512], F32, tag="tps",
                                        name=f"tp{h}_{n}_{j}")
                        nc.tensor.matmul(
                            t_ps[:, 0:33],
                            a_sb[0:33, j * 128:(j + 1) * 128],
                            identb[0:33, 0:33],
                            start=True, stop=True)
                        r_sb = norm.tile([128, 1], F32, tag="r",
                                         name=f"r{h}_{n}_{j}")
                        nc.vector.reciprocal(r_sb[:], t_ps[:, 32:33])
                        z_sb = norm.tile([128, 32], F16, tag="z",
                                         name=f"z{h}_{n}_{j}")
                        nc.vector.tensor_scalar_mul(z_sb[:], t_ps[:, 0:32], r_sb[:])
                        ws = slice(n * 512 + j * 128, n * 512 + j * 128 + 128)
                        nc.sync.dma_start(out=ahwd_d.ap()[h, ws, :], in_=z_sb[:])
                        if n == NWT - 1 and j == 3:
                            # head fully staged -> stream reshape rows in
                            rr = slice((h % 4) * 32, (h % 4) * 32 + 32)
                            nc.sync.dma_start(
                                out=ar_sb[h // 4][rr, :],
                                in_=ar[h * 32:(h + 1) * 32, :])
                    return run

                for g2 in range(4):
                    h0, h1 = 2 * g2, 2 * g2 + 1
                    qt = g2 // 2
                    for n in range(NWT):
                        ns = slice(n * 512, (n + 1) * 512)
                        acc = [aps.tile([128, 512], F32, tag="aps",
                                        name=f"acc{g2}_{n}_{hh}") for hh in range(2)]
                        for m in range(NMT):
                            ms = slice(m * 128, (m + 1) * 128)
                            lg2 = lg.tile([128, 1024], F32, tag="lg",
                                          name=f"lg{g2}_{n}_{m}")
                            for hh, h in ((0, h0), (1, h1)):
                                nc.tensor.matmul(
                                    lg2[:, hh * 512:hh * 512 + 512],
                                    k_sb[qt][:, ms],
                                    qpads[h][:, ns],
                                    start=True, stop=True)
                            e2 = ep.tile([128, 1024], F16, tag="e",
                                         name=f"e{g2}_{n}_{m}")
                            if mstep % 7 in (1, 4):
                                nc.vector.tensor_scalar(
                                    out=e2[:].bitcast(U16), in0=lg2[:],
                                    scalar1=SCHR_A, scalar2=SCHR_B,
                                    op0=mybir.AluOpType.mult,
                                    op1=mybir.AluOpType.add)
                            else:
                                nc.scalar.activation(e2[:], lg2[:],
                                                     mybir.ActivationFunctionType.Exp,
                                                     bias=eshift_sb[:])
                            for hh, h in ((0, h0), (1, h1)):
                                nc.tensor.matmul(
                                    acc[hh][:],
                                    vt_sb[m][:, 33 * h:33 * h + 128],
                                    e2[:, hh * 512:hh * 512 + 512],
                                    start=(m == 0), stop=(m == NMT - 1))
                            if mstep % CONV_EVERY == CONV_EVERY - 1 and cu_i < len(conv_units):
                                conv_unit(*conv_units[cu_i]); cu_i += 1
                            pop_pending(PENDING_PER_STEP)
                            mstep += 1

                        # free accumulators fast: one bf16 copy per head, then
                        # defer transposes+normalize into the next block.
                        for hh, h in ((0, h0), (1, h1)):
                            a_sb = asbp.tile([33, 512], BF16, tag="asb",
                                             name=f"as{h}_{n}")
                            nc.vector.tensor_copy(a_sb[:], acc[hh][0:33, :])
                            for j in range(4):
                                pending.append(make_norm(h, n, a_sb, j))

                        if n == 1 and h1 + 2 < NH:
                            qpads[h0 + 2] = build_qpad(h0 + 2)
                        if n == 2 and h1 + 2 < NH:
                            qpads[h1 + 2] = build_qpad(h1 + 2)

                while pending or cu_i < len(conv_units):
                    if cu_i < len(conv_units):
                        conv_unit(*conv_units[cu_i]); cu_i += 1
                    pop_pending(PENDING_PER_STEP)

                # ---- stage 3: 1x1 conv over the (faithful-reshape) view ----
                wk = lg.tile([128, 512], F32, tag="lg", name="warmkeep")
                for _ in range(6):
                    nc.tensor.matmul(wk[:], wup[:, 0:128], wup[:], start=True, stop=True)
                for t2 in range(2):
                    for n in range(NWT):
                        ps = lg.tile([128, 512], F32, tag="lg", name=f"fin{t2}_{n}")
                        for ct in range(NCT):
                            nc.tensor.matmul(
                                ps[:], watt_sb[ct][:, t2 * 128:(t2 + 1) * 128],
                                ar_sb[ct][:, n * 512:(n + 1) * 512],
                                start=(ct == 0), stop=(ct == NCT - 1))
                        fo = stage.tile([128, 512], F32, tag="fo")
                        nc.vector.tensor_scalar_add(fo[:], ps[:], bat_sb[:, t2:t2 + 1])
                        nc.sync.dma_start(
                            out=out_d.ap()[C + t2 * 128:C + (t2 + 1) * 128,
                                           n * 512:(n + 1) * 512],
                            in_=fo[:])

    nc.compile()
    return nc


_NC_CACHE = []


def _get_nc():
    if not _NC_CACHE:
        _NC_CACHE.append(build_nc())
    return _NC_CACHE[0]


def _prep_in_maps(x, w_conv, b_conv, w_qkv, b_qkv, w_attn, b_attn):
    x = np.asarray(x, np.float16)
    wt = np.ascontiguousarray(
        np.concatenate([np.asarray(w_conv, np.float32), np.asarray(w_qkv, np.float32)], 0)
        .transpose(2, 1, 0).astype(np.float16))                # [3, c, oc]
    b_all = np.concatenate([np.asarray(b_conv, np.float32),
                            np.asarray(b_qkv, np.float32)]).copy()
    b_all[C:2 * C] *= QSCALE                                   # q bias pre-scaled
    watt = np.ascontiguousarray(np.asarray(w_attn, np.float32).T.astype(np.float16))
    bat = np.ascontiguousarray(np.asarray(b_attn, np.float32))
    return [
        {"x": np.ascontiguousarray(x[b]), "wt": wt, "b_all": b_all,
         "watT": watt, "bat": bat}
        for b in range(x.shape[0])
    ]


def run(trace=False, **inputs):
    nc = _get_nc()
    in_maps = _prep_in_maps(**inputs)
    res = bass_utils.run_bass_kernel_spmd(
        nc, in_maps, core_ids=list(range(8)), trace=trace,
        **({"trace_cores": [0]} if trace else {}))
    out = np.stack([res.results[i]["out"] for i in range(8)]).astype(np.float32)
    return out, res


def kernel(**inputs) -> np.ndarray:
    out, _ = run(**inputs)
    return out


# revision 19
# speedup vs baseline: 1.2138x; 1.0307x over previous
"""Trainium2 Bass kernel for nn_AugmentedConv (conv branch + conv-attention branch).

Full-input contract: kernel(**inputs) takes the complete unsharded inputs and
returns the full (8, 512, 2048) output. Internally: data-parallel over batch
across 8 NeuronCores; each core runs the whole module for one batch element.

v2: PE tile_position packing + split exp across ScalarE/DVE.
 - logits: 4 heads concurrent via 32-row tile_position groups (contract=32).
 - attnV: 2 heads per PSUM bank via 64-col tile_position groups; each head
   block in vt is 64 wide (32 v dims + ones col for the softmax denominator).
 - exp: ScalarE exact exp for half the tiles; DVE Schraudolph uint16 bit-trick
   (out = bits(round(x*a+b)) viewed as fp16, saturating: negatives -> +0) for
   the rest. ~3% per-element error on those tiles; cancels largely in softmax.

Hardcoded problem shapes: B=8, C=256, W=2048, DK=DV=256, NH=8, KS=3, pad=1.
"""

import numpy as np

import concourse.bacc as bacc
import concourse.mybir as mybir
import concourse.tile as tile
from concourse import bass_utils
from concourse.masks import make_identity

F32 = mybir.dt.float32
F16 = mybir.dt.float16
BF16 = mybir.dt.bfloat16
U16 = mybir.dt.uint16
ESHIFT = -4.0   # exp(x + ESHIFT): keeps fp16 exp in range; cancels in softmax ratio

C = 256          # input channels
W = 2048         # sequence length
OC = 1024        # combined conv output channels: [conv_out 256 | q 256 | k 256 | v 256]
NH = 8
DKH = 32         # head dim (dk and dv per head)
QSCALE = float(DKH) ** -0.5
NCT = C // 128   # input-channel tiles (2)
NWT = W // 512   # 512-wide w tiles (4)
NMT = W // 128   # 128-wide w tiles (16)

LOG2E = 1.4426950408889634
SCHR_A = 1024.0 * LOG2E
SCHR_B = ESHIFT * SCHR_A + 15 * 1024 - 46.0

# knobs
CONV_EVERY = 14      # emit one conv unit every N m-steps
PENDING_PER_STEP = 3 # deferred normalize items popped per m-step


def build_nc():
    nc = bacc.Bacc("TRN2", target_bir_lowering=False, debug=False)

    x_d = nc.dram_tensor("x", [C, W], F16, kind="ExternalInput")
    wt_d = nc.dram_tensor("wt", [3, C, OC], F16, kind="ExternalInput")     # wt[k,c,oc]
    ball_d = nc.dram_tensor("b_all", [OC], F32, kind="ExternalInput")      # q part pre-scaled
    watt_d = nc.dram_tensor("watT", [C, C], F16, kind="ExternalInput")     # w_attn.T (c,o)
    bat_d = nc.dram_tensor("bat", [C], F32, kind="ExternalInput")
    out_d = nc.dram_tensor("out", [2 * C, W], F32, kind="ExternalOutput")
    # attention output staged in [h, w, d] order; the module's faithful
    # (NH,W,dvh)->(256,W) reshape is then a contiguous view of this buffer.
    ahwd_d = nc.dram_tensor("attn_hwd", [NH, W, DKH], F16)

    with tile.TileContext(nc) as tc:
        import contextlib
        with contextlib.ExitStack() as ctx:
            singles = ctx.enter_context(tc.tile_pool(name="singles", bufs=1))
            xp = ctx.enter_context(tc.tile_pool(name="xp", bufs=NCT))
            wtp = ctx.enter_context(tc.tile_pool(name="wtp", bufs=3 * NCT))
            qkp = ctx.enter_context(tc.tile_pool(name="qkp", bufs=4))
            vtp = ctx.enter_context(tc.tile_pool(name="vtp", bufs=NMT))
            ep = ctx.enter_context(tc.tile_pool(name="ep", bufs=6))
            stage = ctx.enter_context(tc.tile_pool(name="stage", bufs=4))
            norm = ctx.enter_context(tc.tile_pool(name="norm", bufs=10))
            asbp = ctx.enter_context(tc.tile_pool(name="asbp", bufs=4))
            qpp = ctx.enter_context(tc.tile_pool(name="qpp", bufs=4))

            # ---- constants -------------------------------------------------
            ident = singles.tile([128, 128], F32)
            make_identity(nc, ident[:])
            identb = singles.tile([128, 128], BF16)
            nc.vector.tensor_copy(identb[:], ident[:])
            b_sb = singles.tile([128, 8], F32)  # conv biases, [ch-in-tile, oc-tile]
            nc.gpsimd.dma_start(out=b_sb, in_=ball_d.ap().rearrange("(t p) -> p t", p=128))
            bat_sb = singles.tile([128, 2], F32)
            nc.gpsimd.dma_start(out=bat_sb, in_=bat_d.ap().rearrange("(t p) -> p t", p=128))
            bv_sb = singles.tile([128, C], F32)  # v bias replicated across partitions
            nc.gpsimd.dma_start(
                out=bv_sb, in_=ball_d.ap()[3 * C:4 * C].partition_broadcast(128))
            zero1 = singles.tile([128, 1], F32)
            nc.vector.memset(zero1[:], 0.0)
            eshift_sb = singles.tile([128, 1], F32)
            nc.vector.memset(eshift_sb[:], ESHIFT)
            ones8 = singles.tile([128, 8, 1], F32)
            nc.vector.memset(ones8[:], 1.0)
            wup = singles.tile([128, 512], F16)  # PE warm-up fodder
            nc.vector.memset(wup[:], 0.0)

            # ---- load x (zero-padded by one column each side) and weights --
            x_sb = []
            for ct in range(NCT):
                t = xp.tile([128, W + 2], F16, tag="x", name=f"x{ct}")
                nc.vector.tensor_copy(t[:, 0:1], zero1[:])
                nc.vector.tensor_copy(t[:, W + 1:W + 2], zero1[:])
                nc.gpsimd.dma_start(out=t[:, 1:W + 1], in_=x_d.ap()[ct * 128:(ct + 1) * 128, :])
                x_sb.append(t)
            wt_sb = {}
            wtv_sb = {}
            for kk in range(3):
                for ct in range(NCT):
                    t = wtp.tile([128, OC], F16, tag="wt", name=f"wt{kk}_{ct}")
                    nc.gpsimd.dma_start(out=t, in_=wt_d.ap()[kk, ct * 128:(ct + 1) * 128, :])
                    wt_sb[kk, ct] = t
                    wtv_sb[kk, ct] = t[:, 3 * C:4 * C]
            watt_sb = []
            for ct in range(NCT):
                t = qkp.tile([128, C], F16, tag="watt", name=f"watt{ct}")
                nc.gpsimd.dma_start(out=t, in_=watt_d.ap()[ct * 128:(ct + 1) * 128, :])
                watt_sb.append(t)

            # ---- stage 1: q/k convs + vT conv ([ch,W] and [w,64*h] layouts) -
            q_sb, k_sb = [], []
            for qt in range(2):
                q_sb.append(qkp.tile([128, W], F16, tag="qk", name=f"q{qt}"))
                k_sb.append(qkp.tile([128, W], F16, tag="qk", name=f"k{qt}"))

            with tc.tile_pool(name="cps", bufs=4, space="PSUM") as cps:
                # PE warm-up burst: gets HAM to 8/8 while the input DMAs run
                wps = cps.tile([128, 512], F32, tag="cps", name="wps")
                for _ in range(16):
                    nc.tensor.matmul(wps[:], wup[:, 0:128], wup[:], start=True, stop=True)

                # vT conv: [w, vch] layout (x slice is the stationary operand),
                # v bias added here; col 32 of each 33-wide head block stays 1.0
                # (softmax-denominator ones column), cols >=264 zero pad so the
                # 128-col lhsT window at h=7 stays in range.
                vt_sb = []
                for m in range(NMT):
                    vt = vtp.tile([128, NH * 33 + 96], F16, tag="vt", name=f"vt{m}")
                    nc.vector.tensor_copy(
                        vt[:, 0:NH * 33].rearrange("p (h e) -> p h e", e=33)[:, :, 32:33],
                        ones8[:])
                    nc.vector.memset(vt[:, NH * 33:], 0.0)
                    ps = cps.tile([128, C], F32, tag="vps")
                    for ct in range(NCT):
                        for kk in range(3):
                            nc.tensor.matmul(
                                ps[:],
                                x_sb[ct][:, m * 128 + kk:m * 128 + kk + 128],
                                wtv_sb[kk, ct],
                                start=(ct == 0 and kk == 0),
                                stop=(ct == NCT - 1 and kk == 2),
                            )
                    nc.vector.tensor_add(
                        vt[:, 0:NH * 33].rearrange("p (h e) -> p h e", e=33)[:, :, 0:32],
                        ps[:].rearrange("p (h d) -> p h d", d=32),
                        bv_sb[:].rearrange("p (h d) -> p h d", d=32),
                    )
                    vt_sb.append(vt)

                for t in (2, 4):  # oc-tiles: q0, k0 (tile1 deferred)
                    for n in range(NWT):
                        ps = cps.tile([128, 512], F32, tag="cps")
                        for ct in range(NCT):
                            for kk in range(3):
                                nc.tensor.matmul(
                                    ps[:],
                                    wt_sb[kk, ct][:, t * 128:(t + 1) * 128],
                                    x_sb[ct][:, n * 512 + kk:n * 512 + kk + 512],
                                    start=(ct == 0 and kk == 0),
                                    stop=(ct == NCT - 1 and kk == 2),
                                )
                        ns = slice(n * 512, (n + 1) * 512)
                        if t < 4:        # q (scale folded; bias pre-scaled on host)
                            nc.vector.tensor_scalar(
                                out=q_sb[t - 2][:, ns], in0=ps[:],
                                scalar1=QSCALE, scalar2=b_sb[:, t:t + 1],
                                op0=mybir.AluOpType.mult, op1=mybir.AluOpType.add)
                        else:            # k
                            nc.vector.tensor_scalar_add(k_sb[t - 4][:, ns], ps[:], b_sb[:, t:t + 1])

            # ---- stage 2: attention, 2 heads per m-step -------------------
            # Per m-step: one [128,1024] logits pair tile (two FWL-friendly
            # full-contract matmuls against a zero-padded per-head q), one exp
            # (ScalarE exact or DVE Schraudolph by pattern), two attnV matmuls
            # (33-wide vt blocks via 128-col windows, ones col = denominator).
            def build_qpad(h):
                qt = h // 4
                s = 32 * (h % 4)
                qpad = qpp.tile([128, W], F16, tag="qpad", name=f"qpad{h}")
                nc.vector.memset(qpad[:], 0.0)
                nc.vector.tensor_copy(qpad[s:s + 32, :], q_sb[qt][s:s + 32, :])
                return qpad

            qpads = {0: build_qpad(0), 1: build_qpad(1)}

            with tc.tile_pool(name="lg", bufs=2, space="PSUM") as lg, \
                 tc.tile_pool(name="aps", bufs=2, space="PSUM") as aps, \
                 tc.tile_pool(name="tps", bufs=2, space="PSUM") as tps:

                def conv_unit(t, n):
                    ps = tps.tile([128, 512], F32, tag="tps", name=f"cops{t}_{n}")
                    for ct in range(NCT):
                        for kk in range(3):
                            nc.tensor.matmul(
                                ps[:],
                                wt_sb[kk, ct][:, t * 128:(t + 1) * 128],
                                x_sb[ct][:, n * 512 + kk:n * 512 + kk + 512],
                                start=(ct == 0 and kk == 0),
                                stop=(ct == NCT - 1 and kk == 2),
                            )
                    ns = slice(n * 512, (n + 1) * 512)
                    if t < 2:      # conv_out -> DRAM
                        co = stage.tile([128, 512], F32, tag="co")
                        nc.vector.tensor_scalar_add(co[:], ps[:], b_sb[:, t:t + 1])
                        nc.sync.dma_start(out=out_d.ap()[t * 128:(t + 1) * 128, ns], in_=co[:])
                    elif t == 3:   # q tile 1
                        nc.vector.tensor_scalar(
                            out=q_sb[1][:, ns], in0=ps[:],
                            scalar1=QSCALE, scalar2=b_sb[:, t:t + 1],
                            op0=mybir.AluOpType.mult, op1=mybir.AluOpType.add)
                    else:          # k tile 1
                        nc.vector.tensor_scalar_add(k_sb[1][:, ns], ps[:], b_sb[:, t:t + 1])

                # q tile 1 first: build_qpad(4/5) (emitted in g2=1) reads it,
                # and program order is semantic for those stale-read hazards.
                conv_units = ([(3, n) for n in range(NWT)]
                              + [(5, n) for n in range(NWT)]
                              + [(t, n) for t in range(2) for n in range(NWT)])
                cu_i = 0

                # resident reshape source for stage 3: ar[c=(h,g), w2=(r,d)],
                # streamed back per finished head from the [h,w,d] DRAM staging
                ar_sb = [qkp.tile([128, W], F16, tag="ar", name=f"ar{i}") for i in range(2)]
                ar = ahwd_d.ap().rearrange("h (g x) d -> (h g) (x d)", g=32)  # [256, 2048]

                pending = []   # deferred normalize closures
                mstep = 0

                def pop_pending(k):
                    for _ in range(min(k, len(pending))):
                        pending.pop(0)()

                def make_norm(h, n, a_sb, j):
                    # one transpose+normalize chunk: head h, q chunk j
                    def run():
                        t_ps = tps.tile([128, 512], F32, tag="tps",
                                        name=f"tp{h}_{n}_{j}")
                        nc.tensor.matmul(
                            t_ps[:, 0:33],
                            a_sb[0:33, j * 128:(j + 1) * 128],
                            identb[0:33, 0:33],
                            start=True, stop=True)
                        r_sb = norm.tile([128, 1], F32, tag="r",
                                         name=f"r{h}_{n}_{j}")
                        nc.vector.reciprocal(r_sb[:], t_ps[:, 32:33])
                        z_sb = norm.tile([128, 32], F16, tag="z",
                                         name=f"z{h}_{n}_{j}")
                        nc.vector.tensor_scalar_mul(z_sb[:], t_ps[:, 0:32], r_sb[:])
                        ws = slice(n * 512 + j * 128, n * 512 + j * 128 + 128)
                        nc.sync.dma_start(out=ahwd_d.ap()[h, ws, :], in_=z_sb[:])
                        if n == NWT - 1 and j == 3:
                            # head fully staged -> stream reshape rows in
                            rr = slice((h % 4) * 32, (h % 4) * 32 + 32)
                            nc.sync.dma_start(
                                out=ar_sb[h // 4][rr, :],
                                in_=ar[h * 32:(h + 1) * 32, :])
                    return run

                for g2 in range(4):
                    h0, h1 = 2 * g2, 2 * g2 + 1
                    qt = g2 // 2
                    for n in range(NWT):
                        ns = slice(n * 512, (n + 1) * 512)
                        acc = [aps.tile([128, 512], F32, tag="aps",
                                        name=f"acc{g2}_{n}_{hh}") for hh in range(2)]
                        prev_av = None   # attnV deferred one m-step so the PE
                        for m in range(NMT):  # queue never blocks on exp(m)
                            ms = slice(m * 128, (m + 1) * 128)
                            lg2 = lg.tile([128, 1024], F32, tag="lg",
                                          name=f"lg{g2}_{n}_{m}")
                            for hh, h in ((0, h0), (1, h1)):
                                nc.tensor.matmul(
                                    lg2[:, hh * 512:hh * 512 + 512],
                                    k_sb[qt][:, ms],
                                    qpads[h][:, ns],
                                    start=True, stop=True)
                            e2 = ep.tile([128, 1024], F16, tag="e",
                                         name=f"e{g2}_{n}_{m}")
                            if mstep % 7 in (1, 4):
                                nc.vector.tensor_scalar(
                                    out=e2[:].bitcast(U16), in0=lg2[:],
                                    scalar1=SCHR_A, scalar2=SCHR_B,
                                    op0=mybir.AluOpType.mult,
                                    op1=mybir.AluOpType.add)
                            else:
                                nc.scalar.activation(e2[:], lg2[:],
                                                     mybir.ActivationFunctionType.Exp,
                                                     bias=eshift_sb[:])
                            if mstep % CONV_EVERY == CONV_EVERY - 1 and cu_i < len(conv_units):
                                conv_unit(*conv_units[cu_i]); cu_i += 1
                            pop_pending(PENDING_PER_STEP)
                            if prev_av is not None:
                                prev_av()

                            def make_av(m, e2):
                                def run():
                                    for hh, h in ((0, h0), (1, h1)):
                                        nc.tensor.matmul(
                                            acc[hh][:],
                                            vt_sb[m][:, 33 * h:33 * h + 128],
                                            e2[:, hh * 512:hh * 512 + 512],
                                            start=(m == 0), stop=(m == NMT - 1))
                                return run
                            prev_av = make_av(m, e2)
                            mstep += 1
                        prev_av()

                        # free accumulators fast: one bf16 copy per head, then
                        # defer transposes+normalize into the next block.
                        for hh, h in ((0, h0), (1, h1)):
                            a_sb = asbp.tile([33, 512], BF16, tag="asb",
                                             name=f"as{h}_{n}")
                            nc.vector.tensor_copy(a_sb[:], acc[hh][0:33, :])
                            for j in range(4):
                                pending.append(make_norm(h, n, a_sb, j))

                        if n == 1 and h1 + 2 < NH:
                            qpads[h0 + 2] = build_qpad(h0 + 2)
                        if n == 2 and h1 + 2 < NH:
                            qpads[h1 + 2] = build_qpad(h1 + 2)

                while pending or cu_i < len(conv_units):
                    if cu_i < len(conv_units):
                        conv_unit(*conv_units[cu_i]); cu_i += 1
                    pop_pending(PENDING_PER_STEP)

                # ---- stage 3: 1x1 conv over the (faithful-reshape) view ----
                wk = lg.tile([128, 512], F32, tag="lg", name="warmkeep")
                for _ in range(6):
                    nc.tensor.matmul(wk[:], wup[:, 0:128], wup[:], start=True, stop=True)
                for t2 in range(2):
                    for n in range(NWT):
                        ps = lg.tile([128, 512], F32, tag="lg", name=f"fin{t2}_{n}")
                        for ct in range(NCT):
                            nc.tensor.matmul(
                                ps[:], watt_sb[ct][:, t2 * 128:(t2 + 1) * 128],
                                ar_sb[ct][:, n * 512:(n + 1) * 512],
                                start=(ct == 0), stop=(ct == NCT - 1))
                        fo = stage.tile([128, 512], F32, tag="fo")
                        nc.vector.tensor_scalar_add(fo[:], ps[:], bat_sb[:, t2:t2 + 1])
                        nc.sync.dma_start(
                            out=out_d.ap()[C + t2 * 128:C + (t2 + 1) * 128,
                                           n * 512:(n + 1) * 512],
                            in_=fo[:])

    nc.compile()
    return nc


_NC_CACHE = []


def _get_nc():
    if not _NC_CACHE:
        _NC_CACHE.append(build_nc())
    return _NC_CACHE[0]


def _prep_in_maps(x, w_conv, b_conv, w_qkv, b_qkv, w_attn, b_attn):
    x = np.asarray(x, np.float16)
    wt = np.ascontiguousarray(
        np.concatenate([np.asarray(w_conv, np.float32), np.asarray(w_qkv, np.float32)], 0)
        .transpose(2, 1, 0).astype(np.float16))                # [3, c, oc]
    b_all = np.concatenate([np.asarray(b_conv, np.float32),
                            np.asarray(b_qkv, np.float32)]).copy()
    b_all[C:2 * C] *= QSCALE                                   # q bias pre-scaled
    watt = np.ascontiguousarray(np.asarray(w_attn, np.float32).T.astype(np.float16))
    bat = np.ascontiguousarray(np.asarray(b_attn, np.float32))
    return [
        {"x": np.ascontiguousarray(x[b]), "wt": wt, "b_all": b_all,
         "watT": watt, "bat": bat}
        for b in range(x.shape[0])
    ]


def run(trace=False, **inputs):
    nc = _get_nc()
    in_maps = _prep_in_maps(**inputs)
    res = bass_utils.run_bass_kernel_spmd(
        nc, in_maps, core_ids=list(range(8)), trace=trace,
        **({"trace_cores": [0]} if trace else {}))
    out = np.stack([res.results[i]["out"] for i in range(8)]).astype(np.float32)
    return out, res


def kernel(**inputs) -> np.ndarray:
    out, _ = run(**inputs)
    return out


# revision 21
# speedup vs baseline: 1.2171x; 1.0027x over previous
"""Trainium2 Bass kernel for nn_AugmentedConv (conv branch + conv-attention branch).

Full-input contract: kernel(**inputs) takes the complete unsharded inputs and
returns the full (8, 512, 2048) output. Internally: data-parallel over batch
across 8 NeuronCores; each core runs the whole module for one batch element.

v2: PE tile_position packing + split exp across ScalarE/DVE.
 - logits: 4 heads concurrent via 32-row tile_position groups (contract=32).
 - attnV: 2 heads per PSUM bank via 64-col tile_position groups; each head
   block in vt is 64 wide (32 v dims + ones col for the softmax denominator).
 - exp: ScalarE exact exp for half the tiles; DVE Schraudolph uint16 bit-trick
   (out = bits(round(x*a+b)) viewed as fp16, saturating: negatives -> +0) for
   the rest. ~3% per-element error on those tiles; cancels largely in softmax.

Hardcoded problem shapes: B=8, C=256, W=2048, DK=DV=256, NH=8, KS=3, pad=1.
"""

import numpy as np

import concourse.bacc as bacc
import concourse.mybir as mybir
import concourse.tile as tile
from concourse import bass_utils
from concourse.masks import make_identity

F32 = mybir.dt.float32
F16 = mybir.dt.float16
BF16 = mybir.dt.bfloat16
U16 = mybir.dt.uint16
ESHIFT = -4.0   # exp(x + ESHIFT): keeps fp16 exp in range; cancels in softmax ratio

C = 256          # input channels
W = 2048         # sequence length
OC = 1024        # combined conv output channels: [conv_out 256 | q 256 | k 256 | v 256]
NH = 8
DKH = 32         # head dim (dk and dv per head)
QSCALE = float(DKH) ** -0.5
NCT = C // 128   # input-channel tiles (2)
NWT = W // 512   # 512-wide w tiles (4)
NMT = W // 128   # 128-wide w tiles (16)

LOG2E = 1.4426950408889634
SCHR_A = 1024.0 * LOG2E
SCHR_B = ESHIFT * SCHR_A + 15 * 1024 - 46.0

# knobs
CONV_EVERY = 14      # emit one conv unit every N m-steps
PENDING_PER_STEP = 3 # deferred normalize items popped per m-step


def build_nc():
    nc = bacc.Bacc("TRN2", target_bir_lowering=False, debug=False)

    x_d = nc.dram_tensor("x", [C, W], F16, kind="ExternalInput")
    wt_d = nc.dram_tensor("wt", [3, C, OC], F16, kind="ExternalInput")     # wt[k,c,oc]
    ball_d = nc.dram_tensor("b_all", [OC], F32, kind="ExternalInput")      # q part pre-scaled
    watt_d = nc.dram_tensor("watT", [C, C], F16, kind="ExternalInput")     # w_attn.T (c,o)
    bat_d = nc.dram_tensor("bat", [C], F32, kind="ExternalInput")
    out_d = nc.dram_tensor("out", [2 * C, W], F32, kind="ExternalOutput")
    # attention output staged in [h, w, d] order; the module's faithful
    # (NH,W,dvh)->(256,W) reshape is then a contiguous view of this buffer.
    ahwd_d = nc.dram_tensor("attn_hwd", [NH, W, DKH], F16)

    with tile.TileContext(nc) as tc:
        import contextlib
        with contextlib.ExitStack() as ctx:
            singles = ctx.enter_context(tc.tile_pool(name="singles", bufs=1))
            xp = ctx.enter_context(tc.tile_pool(name="xp", bufs=NCT))
            wtp = ctx.enter_context(tc.tile_pool(name="wtp", bufs=3 * NCT))
            qkp = ctx.enter_context(tc.tile_pool(name="qkp", bufs=4))
            vtp = ctx.enter_context(tc.tile_pool(name="vtp", bufs=NMT))
            ep = ctx.enter_context(tc.tile_pool(name="ep", bufs=6))
            stage = ctx.enter_context(tc.tile_pool(name="stage", bufs=4))
            norm = ctx.enter_context(tc.tile_pool(name="norm", bufs=10))
            asbp = ctx.enter_context(tc.tile_pool(name="asbp", bufs=4))
            qpp = ctx.enter_context(tc.tile_pool(name="qpp", bufs=4))

            # ---- constants -------------------------------------------------
            ident = singles.tile([128, 128], F32)
            make_identity(nc, ident[:])
            identb = singles.tile([128, 128], BF16)
            nc.vector.tensor_copy(identb[:], ident[:])
            b_sb = singles.tile([128, 8], F32)  # conv biases, [ch-in-tile, oc-tile]
            nc.gpsimd.dma_start(out=b_sb, in_=ball_d.ap().rearrange("(t p) -> p t", p=128))
            bat_sb = singles.tile([128, 2], F32)
            nc.gpsimd.dma_start(out=bat_sb, in_=bat_d.ap().rearrange("(t p) -> p t", p=128))
            bv_sb = singles.tile([128, C], F32)  # v bias replicated across partitions
            nc.gpsimd.dma_start(
                out=bv_sb, in_=ball_d.ap()[3 * C:4 * C].partition_broadcast(128))
            zero1 = singles.tile([128, 1], F32)
            nc.vector.memset(zero1[:], 0.0)
            eshift_sb = singles.tile([128, 1], F32)
            nc.vector.memset(eshift_sb[:], ESHIFT)
            ones8 = singles.tile([128, 8, 1], F32)
            nc.vector.memset(ones8[:], 1.0)
            wup = singles.tile([128, 512], F16)  # PE warm-up fodder
            nc.vector.memset(wup[:], 0.0)

            # ---- load x (zero-padded by one column each side) and weights --
            x_sb = []
            for ct in range(NCT):
                t = xp.tile([128, W + 2], F16, tag="x", name=f"x{ct}")
                nc.vector.tensor_copy(t[:, 0:1], zero1[:])
                nc.vector.tensor_copy(t[:, W + 1:W + 2], zero1[:])
                nc.gpsimd.dma_start(out=t[:, 1:W + 1], in_=x_d.ap()[ct * 128:(ct + 1) * 128, :])
                x_sb.append(t)
            wt_sb = {}
            wtv_sb = {}
            for kk in range(3):
                for ct in range(NCT):
                    t = wtp.tile([128, OC], F16, tag="wt", name=f"wt{kk}_{ct}")
                    nc.gpsimd.dma_start(out=t, in_=wt_d.ap()[kk, ct * 128:(ct + 1) * 128, :])
                    wt_sb[kk, ct] = t
                    wtv_sb[kk, ct] = t[:, 3 * C:4 * C]
            watt_sb = []
            for ct in range(NCT):
                t = qkp.tile([128, C], F16, tag="watt", name=f"watt{ct}")
                nc.gpsimd.dma_start(out=t, in_=watt_d.ap()[ct * 128:(ct + 1) * 128, :])
                watt_sb.append(t)

            # ---- stage 1: q/k convs + vT conv ([ch,W] and [w,64*h] layouts) -
            q_sb, k_sb = [], []
            for qt in range(2):
                q_sb.append(qkp.tile([128, W], F16, tag="qk", name=f"q{qt}"))
                k_sb.append(qkp.tile([128, W], F16, tag="qk", name=f"k{qt}"))

            with tc.tile_pool(name="cps", bufs=4, space="PSUM") as cps:
                # PE warm-up burst: gets HAM to 8/8 while the input DMAs run
                wps = cps.tile([128, 512], F32, tag="cps", name="wps")
                for _ in range(16):
                    nc.tensor.matmul(wps[:], wup[:, 0:128], wup[:], start=True, stop=True)

                # vT conv: [w, vch] layout (x slice is the stationary operand),
                # v bias added here; col 32 of each 33-wide head block stays 1.0
                # (softmax-denominator ones column), cols >=264 zero pad so the
                # 128-col lhsT window at h=7 stays in range.
                vt_sb = []
                for m in range(NMT):
                    vt = vtp.tile([128, NH * 33 + 96], F16, tag="vt", name=f"vt{m}")
                    nc.vector.tensor_copy(
                        vt[:, 0:NH * 33].rearrange("p (h e) -> p h e", e=33)[:, :, 32:33],
                        ones8[:])
                    nc.vector.memset(vt[:, NH * 33:], 0.0)
                    ps = cps.tile([128, C], F32, tag="vps")
                    for ct in range(NCT):
                        for kk in range(3):
                            nc.tensor.matmul(
                                ps[:],
                                x_sb[ct][:, m * 128 + kk:m * 128 + kk + 128],
                                wtv_sb[kk, ct],
                                start=(ct == 0 and kk == 0),
                                stop=(ct == NCT - 1 and kk == 2),
                            )
                    nc.vector.tensor_add(
                        vt[:, 0:NH * 33].rearrange("p (h e) -> p h e", e=33)[:, :, 0:32],
                        ps[:].rearrange("p (h d) -> p h d", d=32),
                        bv_sb[:].rearrange("p (h d) -> p h d", d=32),
                    )
                    vt_sb.append(vt)

                for t in (2, 4):  # oc-tiles: q0, k0 (tile1 deferred)
                    for n in range(NWT):
                        ps = cps.tile([128, 512], F32, tag="cps")
                        for ct in range(NCT):
                            for kk in range(3):
                                nc.tensor.matmul(
                                    ps[:],
                                    wt_sb[kk, ct][:, t * 128:(t + 1) * 128],
                                    x_sb[ct][:, n * 512 + kk:n * 512 + kk + 512],
                                    start=(ct == 0 and kk == 0),
                                    stop=(ct == NCT - 1 and kk == 2),
                                )
                        ns = slice(n * 512, (n + 1) * 512)
                        if t < 4:        # q (scale folded; bias pre-scaled on host)
                            nc.vector.tensor_scalar(
                                out=q_sb[t - 2][:, ns], in0=ps[:],
                                scalar1=QSCALE, scalar2=b_sb[:, t:t + 1],
                                op0=mybir.AluOpType.mult, op1=mybir.AluOpType.add)
                        else:            # k
                            nc.vector.tensor_scalar_add(k_sb[t - 4][:, ns], ps[:], b_sb[:, t:t + 1])

            # ---- stage 2: attention, 2 heads per m-step -------------------
            # Per m-step: one [128,1024] logits pair tile (two FWL-friendly
            # full-contract matmuls against a zero-padded per-head q), one exp
            # (ScalarE exact or DVE Schraudolph by pattern), two attnV matmuls
            # (33-wide vt blocks via 128-col windows, ones col = denominator).
            def build_qpad(h):
                qt = h // 4
                s = 32 * (h % 4)
                qpad = qpp.tile([128, W], F16, tag="qpad", name=f"qpad{h}")
                nc.vector.memset(qpad[:], 0.0)
                nc.vector.tensor_copy(qpad[s:s + 32, :], q_sb[qt][s:s + 32, :])
                return qpad

            qpads = {0: build_qpad(0), 1: build_qpad(1)}

            with tc.tile_pool(name="lg", bufs=2, space="PSUM") as lg, \
                 tc.tile_pool(name="aps", bufs=2, space="PSUM") as aps, \
                 tc.tile_pool(name="tps", bufs=2, space="PSUM") as tps:

                def conv_unit(t, n):
                    ps = tps.tile([128, 512], F32, tag="tps", name=f"cops{t}_{n}")
                    for ct in range(NCT):
                        for kk in range(3):
                            nc.tensor.matmul(
                                ps[:],
                                wt_sb[kk, ct][:, t * 128:(t + 1) * 128],
                                x_sb[ct][:, n * 512 + kk:n * 512 + kk + 512],
                                start=(ct == 0 and kk == 0),
                                stop=(ct == NCT - 1 and kk == 2),
                            )
                    ns = slice(n * 512, (n + 1) * 512)
                    if t < 2:      # conv_out -> DRAM
                        co = stage.tile([128, 512], F32, tag="co")
                        nc.vector.tensor_scalar_add(co[:], ps[:], b_sb[:, t:t + 1])
                        nc.sync.dma_start(out=out_d.ap()[t * 128:(t + 1) * 128, ns], in_=co[:])
                    elif t == 3:   # q tile 1
                        nc.vector.tensor_scalar(
                            out=q_sb[1][:, ns], in0=ps[:],
                            scalar1=QSCALE, scalar2=b_sb[:, t:t + 1],
                            op0=mybir.AluOpType.mult, op1=mybir.AluOpType.add)
                    else:          # k tile 1
                        nc.vector.tensor_scalar_add(k_sb[1][:, ns], ps[:], b_sb[:, t:t + 1])

                # q tile 1 first: build_qpad(4/5) (emitted in g2=1) reads it,
                # and program order is semantic for those stale-read hazards.
                conv_units = ([(3, n) for n in range(NWT)]
                              + [(5, n) for n in range(NWT)]
                              + [(t, n) for t in range(2) for n in range(NWT)])
                cu_i = 0

                # resident reshape source for stage 3: ar[c=(h,g), w2=(r,d)],
                # streamed back per finished head from the [h,w,d] DRAM staging
                ar_sb = [qkp.tile([128, W], F16, tag="ar", name=f"ar{i}") for i in range(2)]
                ar = ahwd_d.ap().rearrange("h (g x) d -> (h g) (x d)", g=32)  # [256, 2048]

                pending = []   # deferred normalize closures
                mstep = 0

                def pop_pending(k):
                    for _ in range(min(k, len(pending))):
                        pending.pop(0)()

                def make_norm(h, n, a_sb, j):
                    # one transpose+normalize chunk: head h, q chunk j
                    def run():
                        t_ps = tps.tile([128, 512], F32, tag="tps",
                                        name=f"tp{h}_{n}_{j}")
                        nc.tensor.matmul(
                            t_ps[:, 0:33],
                            a_sb[0:33, j * 128:(j + 1) * 128],
                            identb[0:33, 0:33],
                            start=True, stop=True)
                        r_sb = norm.tile([128, 1], F32, tag="r",
                                         name=f"r{h}_{n}_{j}")
                        nc.vector.reciprocal(r_sb[:], t_ps[:, 32:33])
                        z_sb = norm.tile([128, 32], F16, tag="z",
                                         name=f"z{h}_{n}_{j}")
                        nc.vector.tensor_scalar_mul(z_sb[:], t_ps[:, 0:32], r_sb[:])
                        ws = slice(n * 512 + j * 128, n * 512 + j * 128 + 128)
                        nc.sync.dma_start(out=ahwd_d.ap()[h, ws, :], in_=z_sb[:])
                        if j == 3:
                            # this 512-w stretch fully staged -> stream its 8
                            # reshape rows back in (fine-grained: shrinks the
                            # end-of-kernel ar wait)
                            rr = slice((h % 4) * 32 + 8 * n, (h % 4) * 32 + 8 * n + 8)
                            nc.sync.dma_start(
                                out=ar_sb[h // 4][rr, :],
                                in_=ar[h * 32 + 8 * n:h * 32 + 8 * n + 8, :])
                    return run

                for g2 in range(4):
                    h0, h1 = 2 * g2, 2 * g2 + 1
                    qt = g2 // 2
                    for n in range(NWT):
                        ns = slice(n * 512, (n + 1) * 512)
                        acc = [aps.tile([128, 512], F32, tag="aps",
                                        name=f"acc{g2}_{n}_{hh}") for hh in range(2)]
                        prev_av = None   # attnV deferred one m-step so the PE
                        for m in range(NMT):  # queue never blocks on exp(m)
                            ms = slice(m * 128, (m + 1) * 128)
                            lg2 = lg.tile([128, 1024], F32, tag="lg",
                                          name=f"lg{g2}_{n}_{m}")
                            for hh, h in ((0, h0), (1, h1)):
                                nc.tensor.matmul(
                                    lg2[:, hh * 512:hh * 512 + 512],
                                    k_sb[qt][:, ms],
                                    qpads[h][:, ns],
                                    start=True, stop=True)
                            e2 = ep.tile([128, 1024], F16, tag="e",
                                         name=f"e{g2}_{n}_{m}")
                            if mstep % 7 in (1, 4):
                                nc.vector.tensor_scalar(
                                    out=e2[:].bitcast(U16), in0=lg2[:],
                                    scalar1=SCHR_A, scalar2=SCHR_B,
                                    op0=mybir.AluOpType.mult,
                                    op1=mybir.AluOpType.add)
                            else:
                                nc.scalar.activation(e2[:], lg2[:],
                                                     mybir.ActivationFunctionType.Exp,
                                                     bias=eshift_sb[:])
                            if mstep % CONV_EVERY == CONV_EVERY - 1 and cu_i < len(conv_units):
                                conv_unit(*conv_units[cu_i]); cu_i += 1
                            pop_pending(PENDING_PER_STEP)
                            if prev_av is not None:
                                prev_av()

                            def make_av(m, e2):
                                def run():
                                    for hh, h in ((0, h0), (1, h1)):
                                        nc.tensor.matmul(
                                            acc[hh][:],
                                            vt_sb[m][:, 33 * h:33 * h + 128],
                                            e2[:, hh * 512:hh * 512 + 512],
                                            start=(m == 0), stop=(m == NMT - 1))
                                return run
                            prev_av = make_av(m, e2)
                            mstep += 1
                        prev_av()

                        # free accumulators fast: one bf16 copy per head, then
                        # defer transposes+normalize into the next block.
                        for hh, h in ((0, h0), (1, h1)):
                            a_sb = asbp.tile([33, 512], BF16, tag="asb",
                                             name=f"as{h}_{n}")
                            nc.vector.tensor_copy(a_sb[:], acc[hh][0:33, :])
                            for j in range(4):
                                pending.append(make_norm(h, n, a_sb, j))

                        if n == 1 and h1 + 2 < NH:
                            qpads[h0 + 2] = build_qpad(h0 + 2)
                        if n == 2 and h1 + 2 < NH:
                            qpads[h1 + 2] = build_qpad(h1 + 2)

                while pending or cu_i < len(conv_units):
                    if cu_i < len(conv_units):
                        conv_unit(*conv_units[cu_i]); cu_i += 1
                    pop_pending(PENDING_PER_STEP)

                # ---- stage 3: 1x1 conv over the (faithful-reshape) view ----
                wk = lg.tile([128, 512], F32, tag="lg", name="warmkeep")
                for _ in range(6):
                    nc.tensor.matmul(wk[:], wup[:, 0:128], wup[:], start=True, stop=True)
                for t2 in range(2):
                    for n in range(NWT):
                        ps = lg.tile([128, 512], F32, tag="lg", name=f"fin{t2}_{n}")
                        for ct in range(NCT):
                            nc.tensor.matmul(
                                ps[:], watt_sb[ct][:, t2 * 128:(t2 + 1) * 128],
                                ar_sb[ct][:, n * 512:(n + 1) * 512],
                                start=(ct == 0), stop=(ct == NCT - 1))
                        fo = stage.tile([128, 512], F32, tag="fo")
                        nc.vector.tensor_scalar_add(fo[:], ps[:], bat_sb[:, t2:t2 + 1])
                        nc.sync.dma_start(
                            out=out_d.ap()[C + t2 * 128:C + (t2 + 1) * 128,
                                           n * 512:(n + 1) * 512],
                            in_=fo[:])

    nc.compile()
    return nc


_NC_CACHE = []


def _get_nc():
    if not _NC_CACHE:
        _NC_CACHE.append(build_nc())
    return _NC_CACHE[0]


def _prep_in_maps(x, w_conv, b_conv, w_qkv, b_qkv, w_attn, b_attn):
    x = np.asarray(x, np.float16)
    wt = np.ascontiguousarray(
        np.concatenate([np.asarray(w_conv, np.float32), np.asarray(w_qkv, np.float32)], 0)
        .transpose(2, 1, 0).astype(np.float16))                # [3, c, oc]
    b_all = np.concatenate([np.asarray(b_conv, np.float32),
                            np.asarray(b_qkv, np.float32)]).copy()
    b_all[C:2 * C] *= QSCALE                                   # q bias pre-scaled
    watt = np.ascontiguousarray(np.asarray(w_attn, np.float32).T.astype(np.float16))
    bat = np.ascontiguousarray(np.asarray(b_attn, np.float32))
    return [
        {"x": np.ascontiguousarray(x[b]), "wt": wt, "b_all": b_all,
         "watT": watt, "bat": bat}
        for b in range(x.shape[0])
    ]


def run(trace=False, **inputs):
    nc = _get_nc()
    in_maps = _prep_in_maps(**inputs)
    res = bass_utils.run_bass_kernel_spmd(
        nc, in_maps, core_ids=list(range(8)), trace=trace,
        **({"trace_cores": [0]} if trace else {}))
    out = np.stack([res.results[i]["out"] for i in range(8)]).astype(np.float32)
    return out, res


def kernel(**inputs) -> np.ndarray:
    out, _ = run(**inputs)
    return out
